# revision 34
# baseline (speedup 1.0000x reference)
"""Trainium2 Bass kernel for gumbel-softmax attention.

Reference computation (all f32):
    scores = Q @ K.T / sqrt(64)            # [16384, 4096]
    p      = softmax(scores + g, axis=-1)  # g: fixed Gumbel noise, key 42
    out    = p @ V                         # [16384, 1024]
    return (out, p)

Strategy: shard genes (rows of Q) over 8 cores, 2048 genes/core; K, V
replicated.  Everything on-device is computed in TRANSPOSED layout
[latent, genes] so the latent (contraction) dim lands on SBUF partitions
for both matmuls -- no on-device transposes needed:

    scoresT = KT_chunk.T @ QT              (PE, bf16)
    logitsT = scoresT + gT                 (DVE, fp16 noise, in-place PSUM)
    expT    = exp(logitsT)                 (ACT, -> bf16; no max-sub needed:
                                            logits <= ~25, exp fits f32)
    rowsumT = ones.T @ expT                (PE, [1, genes] accumulated)
    out_un  = expT.T @ V                   (PE accumulate, copied out via ACT)

The device ships expT (bf16), out_un and rowsums; the HOST applies the
softmax normalization (a per-gene f32 scale -- bit-identical math to
doing it on-device, but halves the p-output traffic and removes all
cross-engine normalization dependencies from the device pipeline).

The Gumbel noise is input-independent (fixed key) -> precomputed on host
with the same jax call the reference makes (this environment uses the
rbg PRNG whose bits are backend-dependent -- must run on the default
backend) and streamed in as fp16.
"""

import os
import numpy as np

N_GENES, N_LATENT, D_K, N_CELLS = 16384, 4096, 64, 1024
N_CORES = 8
GPC = N_GENES // N_CORES      # genes per core = 2048
GB = 512                      # gene block width
NBLK = GPC // GB              # gene blocks per core = 4
NSB = GB // 128               # 128-gene sub-blocks per block = 4
NLT = N_LATENT // 128         # latent tiles = 32

_CACHE = {}


def _gumbel_noise():
    """Bit-exact reproduction of the reference's fixed Gumbel noise.

    Must run on the DEFAULT jax backend with default PRNG config: this
    environment uses `jax_default_prng_impl=rbg`, whose bits are
    backend-dependent, and the reference computes on the default device.
    """
    import jax
    import jax.numpy as jnp

    key = jax.random.key(42)
    u = jax.random.uniform(
        key, (N_GENES, N_LATENT), dtype=jnp.float32,
        minval=float(np.finfo(np.float32).tiny), maxval=1.0,
    )
    return np.asarray(-jnp.log(-jnp.log(u)))


def _build_bass():
    import contextlib
    import concourse.bacc as bacc
    import concourse.tile as tile
    from concourse import mybir

    f32 = mybir.dt.float32
    bf16 = mybir.dt.bfloat16
    f16 = mybir.dt.float16

    nc = bacc.Bacc("TRN2", target_bir_lowering=False, debug=False,
                   num_devices=N_CORES)

    # qt/kt ship with rows duplicated into partitions 64..127 so two K=64
    # scores matmuls can run concurrently in the PE array's two row-halves.
    qt = nc.dram_tensor("qt", [2 * D_K, GPC], bf16, kind="ExternalInput").ap()
    kt = nc.dram_tensor("kt", [2 * D_K, N_LATENT], bf16, kind="ExternalInput").ap()
    vp = nc.dram_tensor("vp", [N_LATENT, N_CELLS], bf16, kind="ExternalInput").ap()
    gt = nc.dram_tensor("gt", [N_LATENT, GPC], f16, kind="ExternalInput").ap()
    pt = nc.dram_tensor("pt", [N_LATENT, GPC], bf16, kind="ExternalOutput").ap()
    outp = nc.dram_tensor("outp", [GPC, N_CELLS], f32, kind="ExternalOutput").ap()

    Exp = mybir.ActivationFunctionType.Exp

    with tile.TileContext(nc) as tc, contextlib.ExitStack() as ctx:
        consts = ctx.enter_context(tc.tile_pool(name="consts", bufs=1))
        gpool = ctx.enter_context(tc.tile_pool(name="gpool", bufs=8))
        epool = ctx.enter_context(tc.tile_pool(name="epool", bufs=2))
        outpool = ctx.enter_context(tc.tile_pool(name="outpool", bufs=3))
        lpool = ctx.enter_context(tc.tile_pool(name="lpool", bufs=4))
        scpool = ctx.enter_context(tc.tile_pool(name="scpool", bufs=2, space="PSUM"))
        popool = ctx.enter_context(tc.tile_pool(name="popool", bufs=2, space="PSUM"))

        # Load only what the first matmuls need up front; the rest of the
        # constant loads are deferred into block 0's step loop so the
        # critical first chunks don't contend for DMA bandwidth.
        qt_s = consts.tile([2 * D_K, GPC], bf16)
        nc.sync.dma_start(out=qt_s[:, 0:GB], in_=qt[:, 0:GB])
        kt_s = consts.tile([2 * D_K, N_LATENT], bf16)
        nc.sync.dma_start(out=kt_s[:, 0:GB], in_=kt[:, 0:GB])
        vp_s = consts.tile([128, NLT, N_CELLS], bf16)
        vp_r = vp.rearrange("(t p) c -> p t c", p=128)

        def deferred_const_loads():
            for j in range(1, NBLK):
                yield lambda j=j: nc.sync.dma_start(
                    out=qt_s[:, j * GB:(j + 1) * GB], in_=qt[:, j * GB:(j + 1) * GB])
            for j in range(1, N_LATENT // GB):
                yield lambda j=j: nc.sync.dma_start(
                    out=kt_s[:, j * GB:(j + 1) * GB], in_=kt[:, j * GB:(j + 1) * GB])

        _deferred = deferred_const_loads()

        exps_t = [None] * NBLK
        PREF = 2  # gt prefetch distance, in packed (2-tile) steps

        def phase_a_steps(b):
            """Packed steps for block b, two latent tiles at a time:
            noise DMA (prefetched), 2 concurrent scores MMs (row-halves
            of the PE array), +noise (DVE), exp (ACT), expT DMA out (on
            GpSimd queues -- Sync is the busier DMA issuer)."""
            exps = epool.tile([128, NLT, GB], bf16, name="exps")
            exps_t[b] = exps
            gtiles = {}

            def fetch(k):
                if k >= NLT // 2:
                    return
                gti = gpool.tile([128, 2 * GB], f16, name="gti")
                i = 2 * k
                nc.sync.dma_start(
                    out=gti[:, 0:GB],
                    in_=gt[i * 128:(i + 1) * 128, b * GB:(b + 1) * GB])
                nc.sync.dma_start(
                    out=gti[:, GB:2 * GB],
                    in_=gt[(i + 1) * 128:(i + 2) * 128, b * GB:(b + 1) * GB])
                gtiles[k] = gti

            for k in range(PREF):
                fetch(k)
            for k in range(NLT // 2):
                i = 2 * k
                fetch(k + PREF)
                if b == 0:
                    # hide the remaining const loads + V preload behind
                    # block 0's noise stream
                    for fn in (next(_deferred, None), next(_deferred, None)):
                        if fn is not None:
                            fn()
                    nc.sync.dma_start(out=vp_s[:, i, :], in_=vp_r[:, i, :])
                    nc.sync.dma_start(out=vp_s[:, i + 1, :], in_=vp_r[:, i + 1, :])
                gti = gtiles.pop(k)
                sc = scpool.tile([128, 2 * GB], f32, name="sc")
                nc.tensor.matmul(
                    sc[:, 0:GB], lhsT=kt_s[0:D_K, i * 128:(i + 1) * 128],
                    rhs=qt_s[0:D_K, b * GB:(b + 1) * GB], start=True, stop=True)
                nc.tensor.matmul(
                    sc[:, GB:2 * GB],
                    lhsT=kt_s[D_K:2 * D_K, (i + 1) * 128:(i + 2) * 128],
                    rhs=qt_s[D_K:2 * D_K, b * GB:(b + 1) * GB],
                    start=True, stop=True)
                ltile = lpool.tile([128, 2 * GB], f32, name="ltile")
                nc.vector.tensor_add(ltile, sc, gti)
                nc.scalar.activation(out=exps[:, i:i + 2, :], in_=ltile, func=Exp)
                pt_view = pt[i * 128:(i + 2) * 128, b * GB:(b + 1) * GB] \
                    .rearrange("(t p) g -> p t g", p=128)
                nc.gpsimd.dma_start(out=pt_view, in_=exps[:, i:i + 2, :])
                yield

        def out_mms(b, s, i, po):
            lhs = exps_t[b][:, i, s * 128:(s + 1) * 128]
            nc.tensor.matmul(po[:, 0:512], lhsT=lhs, rhs=vp_s[:, i, 0:512],
                             start=(i == 0), stop=(i == NLT - 1))
            nc.tensor.matmul(po[:, 512:1024], lhsT=lhs, rhs=vp_s[:, i, 512:1024],
                             start=(i == 0), stop=(i == NLT - 1))

        def emit_outn(b, s, po):
            outn = outpool.tile([128, N_CELLS], f32, name="outn")
            nc.scalar.copy(out=outn[:, 0:512], in_=po[:, 0:512])
            nc.vector.tensor_copy(out=outn[:, 512:1024], in_=po[:, 512:1024])
            g0 = (b * NSB + s) * 128
            nc.sync.dma_start(out=outp[g0:g0 + 128, :], in_=outn)

        def emit_phase_b(b, subs, a_next, period):
            """out_un = expT.T @ V per 128-gene sub-block, with the next
            block's phase-A steps interleaved so DVE/ACT chew block b+1
            while PE runs block b's matmuls."""
            for s in subs:
                po = popool.tile([128, N_CELLS], f32, name="po")
                for i in range(NLT):
                    out_mms(b, s, i, po)
                    if a_next is not None and i % period == 1:
                        next(a_next, None)
                emit_outn(b, s, po)
            if a_next is not None:
                for _ in a_next:
                    pass

        # Software pipeline.  Block 0's phase A is DMA-bound (noise + V
        # preload) with PE nearly idle, so sub-block 0's out-matmuls are
        # interleaved directly into it, lagging the exp stream by 2 steps.
        a0 = phase_a_steps(0)
        next(a0)
        next(a0)
        po0 = popool.tile([128, N_CELLS], f32, name="po")
        for k in range(2, NLT // 2):
            next(a0, None)
            for i in (2 * (k - 2), 2 * (k - 2) + 1):
                out_mms(0, 0, i, po0)
        for _ in a0:
            pass
        for i in range(NLT - 4, NLT):
            out_mms(0, 0, i, po0)
        emit_outn(0, 0, po0)

        emit_phase_b(0, range(1, NSB), phase_a_steps(1), period=6)
        for b in range(1, NBLK):
            a_next = phase_a_steps(b + 1) if b + 1 < NBLK else None
            emit_phase_b(b, range(NSB), a_next, period=8)

    nc.compile()
    return nc


def _prep_inputs(query, key, value):
    import ml_dtypes

    bf16 = ml_dtypes.bfloat16
    if "gt" not in _CACHE:
        g = _gumbel_noise()
        gtf = g.T.astype(np.float16)                             # [4096, 16384]
        _CACHE["gt"] = [np.ascontiguousarray(gtf[:, c * GPC:(c + 1) * GPC])
                        for c in range(N_CORES)]
    qt_full = (query.astype(np.float32).T / 8.0).astype(bf16)   # [64, 16384]
    qt_full = np.vstack([qt_full, qt_full])                      # [128, 16384]
    kt1 = np.ascontiguousarray(key.astype(np.float32).T).astype(bf16)
    kt = np.vstack([kt1, kt1])                                   # [128, 4096]
    vp = value.astype(bf16)

    in_maps = []
    for c in range(N_CORES):
        sl = slice(c * GPC, (c + 1) * GPC)
        in_maps.append({
            "qt": np.ascontiguousarray(qt_full[:, sl]),
            "kt": kt,
            "vp": vp,
            "gt": _CACHE["gt"][c],
        })
    return in_maps


LAST_RESULT = None


def _ensure_ntff_hook():
    """Make trace=True usable even when the image lacks antenv.axon_hooks
    (recreate the module and register the ctypes NTFF hook)."""
    import sys, types
    try:
        import antenv.axon_hooks  # noqa: F401
        return
    except ImportError:
        pass
    try:
        import antenv
        from trn_agent_boot.trn_boot import _ntff_profile_via_ctypes
        mod = types.ModuleType("antenv.axon_hooks")
        _h = [None]
        mod.set_axon_ntff_profile_hook = lambda h: _h.__setitem__(0, h)
        mod.get_axon_ntff_profile_hook = lambda: _h[0]
        sys.modules["antenv.axon_hooks"] = mod
        antenv.axon_hooks = mod
        mod.set_axon_ntff_profile_hook(
            _ntff_profile_via_ctypes("/opt/axon/libaxon_pjrt.so"))
    except Exception:
        pass


def kernel(query, key, value):
    global LAST_RESULT
    from concourse.bass_utils import run_bass_kernel_spmd

    trace = bool(int(os.environ.get("KERNEL_TRACE", "0")))
    if trace:
        _ensure_ntff_hook()
    if "nc" not in _CACHE:
        _CACHE["nc"] = _build_bass()
    nc = _CACHE["nc"]

    in_maps = _prep_inputs(np.asarray(query), np.asarray(key), np.asarray(value))
    res = run_bass_kernel_spmd(
        nc, in_maps, core_ids=list(range(N_CORES)), trace=trace)
    LAST_RESULT = res

    out = np.empty((N_GENES, N_CELLS), np.float32)
    p = np.empty((N_GENES, N_LATENT), np.float32)
    for c in range(N_CORES):
        sl = slice(c * GPC, (c + 1) * GPC)
        r = res.results[c]
        ptf = r["pt"].astype(np.float32)          # [latent, genes]
        recip = 1.0 / ptf.sum(axis=0)             # softmax denominators
        out[sl] = r["outp"] * recip[:, None]
        # p[g, l] = expT[l, g] * recip[g]
        p[sl] = (ptf * recip[None, :]).T
    return out, p


# revision 36
# speedup vs baseline: 1.0096x; 1.0096x over previous
"""Trainium2 Bass kernel for gumbel-softmax attention.

Reference computation (all f32):
    scores = Q @ K.T / sqrt(64)            # [16384, 4096]
    p      = softmax(scores + g, axis=-1)  # g: fixed Gumbel noise, key 42
    out    = p @ V                         # [16384, 1024]
    return (out, p)

Strategy: shard genes (rows of Q) over 8 cores, 2048 genes/core; K, V
replicated.  Everything on-device is computed in TRANSPOSED layout
[latent, genes] so the latent (contraction) dim lands on SBUF partitions
for both matmuls -- no on-device transposes needed:

    scoresT = KT_chunk.T @ QT              (PE, bf16)
    logitsT = scoresT + gT                 (DVE, fp16 noise, in-place PSUM)
    expT    = exp(logitsT)                 (ACT, -> bf16; no max-sub needed:
                                            logits <= ~25, exp fits f32)
    rowsumT = ones.T @ expT                (PE, [1, genes] accumulated)
    out_un  = expT.T @ V                   (PE accumulate, copied out via ACT)

The device ships expT (bf16), out_un and rowsums; the HOST applies the
softmax normalization (a per-gene f32 scale -- bit-identical math to
doing it on-device, but halves the p-output traffic and removes all
cross-engine normalization dependencies from the device pipeline).

The Gumbel noise is input-independent (fixed key) -> precomputed on host
with the same jax call the reference makes (this environment uses the
rbg PRNG whose bits are backend-dependent -- must run on the default
backend) and streamed in as fp16.
"""

import os
import numpy as np

N_GENES, N_LATENT, D_K, N_CELLS = 16384, 4096, 64, 1024
N_CORES = 8
GPC = N_GENES // N_CORES      # genes per core = 2048
GB = 512                      # gene block width
NBLK = GPC // GB              # gene blocks per core = 4
NSB = GB // 128               # 128-gene sub-blocks per block = 4
NLT = N_LATENT // 128         # latent tiles = 32

_CACHE = {}


def _gumbel_noise():
    """Bit-exact reproduction of the reference's fixed Gumbel noise.

    Must run on the DEFAULT jax backend with default PRNG config: this
    environment uses `jax_default_prng_impl=rbg`, whose bits are
    backend-dependent, and the reference computes on the default device.
    """
    import jax
    import jax.numpy as jnp

    key = jax.random.key(42)
    u = jax.random.uniform(
        key, (N_GENES, N_LATENT), dtype=jnp.float32,
        minval=float(np.finfo(np.float32).tiny), maxval=1.0,
    )
    return np.asarray(-jnp.log(-jnp.log(u)))


def _build_bass():
    import contextlib
    import concourse.bacc as bacc
    import concourse.tile as tile
    from concourse import mybir

    f32 = mybir.dt.float32
    bf16 = mybir.dt.bfloat16
    f16 = mybir.dt.float16

    nc = bacc.Bacc("TRN2", target_bir_lowering=False, debug=False,
                   num_devices=N_CORES)

    # qt/kt ship with rows duplicated into partitions 64..127 so two K=64
    # scores matmuls can run concurrently in the PE array's two row-halves.
    qt = nc.dram_tensor("qt", [2 * D_K, GPC], bf16, kind="ExternalInput").ap()
    kt = nc.dram_tensor("kt", [2 * D_K, N_LATENT], bf16, kind="ExternalInput").ap()
    vp = nc.dram_tensor("vp", [N_LATENT, N_CELLS], bf16, kind="ExternalInput").ap()
    gt = nc.dram_tensor("gt", [N_LATENT, GPC], f16, kind="ExternalInput").ap()
    pt = nc.dram_tensor("pt", [N_LATENT, GPC], bf16, kind="ExternalOutput").ap()
    outp = nc.dram_tensor("outp", [GPC, N_CELLS], f32, kind="ExternalOutput").ap()

    Exp = mybir.ActivationFunctionType.Exp

    with tile.TileContext(nc) as tc, contextlib.ExitStack() as ctx:
        consts = ctx.enter_context(tc.tile_pool(name="consts", bufs=1))
        gpool = ctx.enter_context(tc.tile_pool(name="gpool", bufs=8))
        epool = ctx.enter_context(tc.tile_pool(name="epool", bufs=2))
        outpool = ctx.enter_context(tc.tile_pool(name="outpool", bufs=3))
        lpool = ctx.enter_context(tc.tile_pool(name="lpool", bufs=4))
        scpool = ctx.enter_context(tc.tile_pool(name="scpool", bufs=2, space="PSUM"))
        popool = ctx.enter_context(tc.tile_pool(name="popool", bufs=2, space="PSUM"))

        # Load only what the first matmuls need up front; the rest of the
        # constant loads are deferred into block 0's step loop so the
        # critical first chunks don't contend for DMA bandwidth.
        qt_s = consts.tile([2 * D_K, GPC], bf16)
        nc.sync.dma_start(out=qt_s[:, 0:GB], in_=qt[:, 0:GB])
        kt_s = consts.tile([2 * D_K, N_LATENT], bf16)
        nc.sync.dma_start(out=kt_s[:, 0:256], in_=kt[:, 0:256])
        nc.sync.dma_start(out=kt_s[:, 256:GB], in_=kt[:, 256:GB])
        vp_s = consts.tile([128, NLT, N_CELLS], bf16)
        vp_r = vp.rearrange("(t p) c -> p t c", p=128)

        def deferred_const_loads():
            for j in range(1, NBLK):
                yield lambda j=j: nc.sync.dma_start(
                    out=qt_s[:, j * GB:(j + 1) * GB], in_=qt[:, j * GB:(j + 1) * GB])
            for j in range(1, N_LATENT // GB):
                yield lambda j=j: nc.sync.dma_start(
                    out=kt_s[:, j * GB:(j + 1) * GB], in_=kt[:, j * GB:(j + 1) * GB])

        _deferred = deferred_const_loads()

        exps_t = [None] * NBLK
        PREF = 2  # gt prefetch distance, in packed (2-tile) steps

        def phase_a_steps(b):
            """Packed steps for block b, two latent tiles at a time:
            noise DMA (prefetched), 2 concurrent scores MMs (row-halves
            of the PE array), +noise (DVE), exp (ACT), expT DMA out (on
            GpSimd queues -- Sync is the busier DMA issuer)."""
            exps = epool.tile([128, NLT, GB], bf16, name="exps")
            exps_t[b] = exps
            gtiles = {}

            def fetch(k):
                if k >= NLT // 2:
                    return
                gti = gpool.tile([128, 2 * GB], f16, name="gti")
                i = 2 * k
                nc.sync.dma_start(
                    out=gti[:, 0:GB],
                    in_=gt[i * 128:(i + 1) * 128, b * GB:(b + 1) * GB])
                nc.sync.dma_start(
                    out=gti[:, GB:2 * GB],
                    in_=gt[(i + 1) * 128:(i + 2) * 128, b * GB:(b + 1) * GB])
                gtiles[k] = gti

            for k in range(PREF):
                fetch(k)
            for k in range(NLT // 2):
                i = 2 * k
                fetch(k + PREF)
                if b == 0:
                    # hide the remaining const loads + V preload behind
                    # block 0's noise stream
                    for fn in (next(_deferred, None), next(_deferred, None)):
                        if fn is not None:
                            fn()
                    nc.sync.dma_start(out=vp_s[:, i, :], in_=vp_r[:, i, :])
                    nc.sync.dma_start(out=vp_s[:, i + 1, :], in_=vp_r[:, i + 1, :])
                gti = gtiles.pop(k)
                sc = scpool.tile([128, 2 * GB], f32, name="sc")
                nc.tensor.matmul(
                    sc[:, 0:GB], lhsT=kt_s[0:D_K, i * 128:(i + 1) * 128],
                    rhs=qt_s[0:D_K, b * GB:(b + 1) * GB], start=True, stop=True)
                nc.tensor.matmul(
                    sc[:, GB:2 * GB],
                    lhsT=kt_s[D_K:2 * D_K, (i + 1) * 128:(i + 2) * 128],
                    rhs=qt_s[D_K:2 * D_K, b * GB:(b + 1) * GB],
                    start=True, stop=True)
                es = lpool.tile([128, 2 * GB], f32, name="es")
                nc.scalar.activation(out=es, in_=sc, func=Exp)
                # exp(s+g) = exp(s) * expg; alternate DVE / GpSimd
                eng = nc.vector if k % 2 == 0 else nc.gpsimd
                eng.tensor_mul(exps[:, i:i + 2, :], es, gti)
                pt_view = pt[i * 128:(i + 2) * 128, b * GB:(b + 1) * GB] \
                    .rearrange("(t p) g -> p t g", p=128)
                nc.gpsimd.dma_start(out=pt_view, in_=exps[:, i:i + 2, :])
                yield

        def out_mms(b, s, i, po):
            lhs = exps_t[b][:, i, s * 128:(s + 1) * 128]
            nc.tensor.matmul(po[:, 0:512], lhsT=lhs, rhs=vp_s[:, i, 0:512],
                             start=(i == 0), stop=(i == NLT - 1))
            nc.tensor.matmul(po[:, 512:1024], lhsT=lhs, rhs=vp_s[:, i, 512:1024],
                             start=(i == 0), stop=(i == NLT - 1))

        def emit_outn(b, s, po):
            outn = outpool.tile([128, N_CELLS], f32, name="outn")
            nc.scalar.copy(out=outn[:, 0:512], in_=po[:, 0:512])
            nc.vector.tensor_copy(out=outn[:, 512:1024], in_=po[:, 512:1024])
            g0 = (b * NSB + s) * 128
            nc.gpsimd.dma_start(out=outp[g0:g0 + 128, :], in_=outn)

        def emit_phase_b(b, subs, a_next, period):
            """out_un = expT.T @ V per 128-gene sub-block, with the next
            block's phase-A steps interleaved so DVE/ACT chew block b+1
            while PE runs block b's matmuls."""
            for s in subs:
                po = popool.tile([128, N_CELLS], f32, name="po")
                for i in range(NLT):
                    out_mms(b, s, i, po)
                    if a_next is not None and i % period == 1:
                        next(a_next, None)
                emit_outn(b, s, po)
            if a_next is not None:
                for _ in a_next:
                    pass

        # Software pipeline.  Block 0's phase A is DMA-bound (noise + V
        # preload) with PE nearly idle, so sub-block 0's out-matmuls are
        # interleaved directly into it, lagging the exp stream by 2 steps.
        a0 = phase_a_steps(0)
        next(a0)
        next(a0)
        po0 = popool.tile([128, N_CELLS], f32, name="po")
        for k in range(2, NLT // 2):
            next(a0, None)
            for i in (2 * (k - 2), 2 * (k - 2) + 1):
                out_mms(0, 0, i, po0)
        for _ in a0:
            pass
        for i in range(NLT - 4, NLT):
            out_mms(0, 0, i, po0)
        emit_outn(0, 0, po0)

        emit_phase_b(0, range(1, NSB), phase_a_steps(1), period=6)
        for b in range(1, NBLK):
            a_next = phase_a_steps(b + 1) if b + 1 < NBLK else None
            emit_phase_b(b, range(NSB), a_next, period=8)

    nc.compile()
    return nc


def _prep_inputs(query, key, value):
    import ml_dtypes

    bf16 = ml_dtypes.bfloat16
    if "gt" not in _CACHE:
        g = _gumbel_noise()
        # ship exp(g)*2^-8 in fp16 (exp-domain: uniform 2^-11 relative
        # error; the 2^-8 scale keeps it in fp16 range and cancels in the
        # host-side softmax normalization)
        egt = np.exp(g.T - np.float32(8 * np.log(2))).astype(np.float16)
        _CACHE["gt"] = [np.ascontiguousarray(egt[:, c * GPC:(c + 1) * GPC])
                        for c in range(N_CORES)]
    qt_full = (query.astype(np.float32).T / 8.0).astype(bf16)   # [64, 16384]
    qt_full = np.vstack([qt_full, qt_full])                      # [128, 16384]
    kt1 = np.ascontiguousarray(key.astype(np.float32).T).astype(bf16)
    kt = np.vstack([kt1, kt1])                                   # [128, 4096]
    vp = value.astype(bf16)

    in_maps = []
    for c in range(N_CORES):
        sl = slice(c * GPC, (c + 1) * GPC)
        in_maps.append({
            "qt": np.ascontiguousarray(qt_full[:, sl]),
            "kt": kt,
            "vp": vp,
            "gt": _CACHE["gt"][c],
        })
    return in_maps


LAST_RESULT = None


def _ensure_ntff_hook():
    """Make trace=True usable even when the image lacks antenv.axon_hooks
    (recreate the module and register the ctypes NTFF hook)."""
    import sys, types
    try:
        import antenv.axon_hooks  # noqa: F401
        return
    except ImportError:
        pass
    try:
        import antenv
        from trn_agent_boot.trn_boot import _ntff_profile_via_ctypes
        mod = types.ModuleType("antenv.axon_hooks")
        _h = [None]
        mod.set_axon_ntff_profile_hook = lambda h: _h.__setitem__(0, h)
        mod.get_axon_ntff_profile_hook = lambda: _h[0]
        sys.modules["antenv.axon_hooks"] = mod
        antenv.axon_hooks = mod
        mod.set_axon_ntff_profile_hook(
            _ntff_profile_via_ctypes("/opt/axon/libaxon_pjrt.so"))
    except Exception:
        pass


def kernel(query, key, value):
    global LAST_RESULT
    from concourse.bass_utils import run_bass_kernel_spmd

    trace = bool(int(os.environ.get("KERNEL_TRACE", "0")))
    if trace:
        _ensure_ntff_hook()
    if "nc" not in _CACHE:
        _CACHE["nc"] = _build_bass()
    nc = _CACHE["nc"]

    in_maps = _prep_inputs(np.asarray(query), np.asarray(key), np.asarray(value))
    res = run_bass_kernel_spmd(
        nc, in_maps, core_ids=list(range(N_CORES)), trace=trace)
    LAST_RESULT = res

    out = np.empty((N_GENES, N_CELLS), np.float32)
    p = np.empty((N_GENES, N_LATENT), np.float32)
    for c in range(N_CORES):
        sl = slice(c * GPC, (c + 1) * GPC)
        r = res.results[c]
        ptf = r["pt"].astype(np.float32)          # [latent, genes]
        recip = 1.0 / ptf.sum(axis=0)             # softmax denominators
        out[sl] = r["outp"] * recip[:, None]
        # p[g, l] = expT[l, g] * recip[g]
        p[sl] = (ptf * recip[None, :]).T
    return out, p


# revision 39
# speedup vs baseline: 1.0219x; 1.0122x over previous
"""Trainium2 Bass kernel for gumbel-softmax attention.

Reference computation (all f32):
    scores = Q @ K.T / sqrt(64)            # [16384, 4096]
    p      = softmax(scores + g, axis=-1)  # g: fixed Gumbel noise, key 42
    out    = p @ V                         # [16384, 1024]
    return (out, p)

Strategy: shard genes (rows of Q) over 8 cores, 2048 genes/core; K, V
replicated.  Everything on-device is computed in TRANSPOSED layout
[latent, genes] so the latent (contraction) dim lands on SBUF partitions
for both matmuls -- no on-device transposes needed:

    scoresT = KT_chunk.T @ QT              (PE, bf16)
    logitsT = scoresT + gT                 (DVE, fp16 noise, in-place PSUM)
    expT    = exp(logitsT)                 (ACT, -> bf16; no max-sub needed:
                                            logits <= ~25, exp fits f32)
    rowsumT = ones.T @ expT                (PE, [1, genes] accumulated)
    out_un  = expT.T @ V                   (PE accumulate, copied out via ACT)

The device ships expT (bf16), out_un and rowsums; the HOST applies the
softmax normalization (a per-gene f32 scale -- bit-identical math to
doing it on-device, but halves the p-output traffic and removes all
cross-engine normalization dependencies from the device pipeline).

The Gumbel noise is input-independent (fixed key) -> precomputed on host
with the same jax call the reference makes (this environment uses the
rbg PRNG whose bits are backend-dependent -- must run on the default
backend) and streamed in as fp16.
"""

import os
import numpy as np

N_GENES, N_LATENT, D_K, N_CELLS = 16384, 4096, 64, 1024
N_CORES = 8
GPC = N_GENES // N_CORES      # genes per core = 2048
GB = 512                      # gene block width
NBLK = GPC // GB              # gene blocks per core = 4
NSB = GB // 128               # 128-gene sub-blocks per block = 4
NLT = N_LATENT // 128         # latent tiles = 32

_CACHE = {}


def _gumbel_noise():
    """Bit-exact reproduction of the reference's fixed Gumbel noise.

    Must run on the DEFAULT jax backend with default PRNG config: this
    environment uses `jax_default_prng_impl=rbg`, whose bits are
    backend-dependent, and the reference computes on the default device.
    """
    import jax
    import jax.numpy as jnp

    key = jax.random.key(42)
    u = jax.random.uniform(
        key, (N_GENES, N_LATENT), dtype=jnp.float32,
        minval=float(np.finfo(np.float32).tiny), maxval=1.0,
    )
    return np.asarray(-jnp.log(-jnp.log(u)))


def _build_bass():
    import contextlib
    import concourse.bacc as bacc
    import concourse.tile as tile
    from concourse import mybir

    f32 = mybir.dt.float32
    bf16 = mybir.dt.bfloat16
    f16 = mybir.dt.float16

    nc = bacc.Bacc("TRN2", target_bir_lowering=False, debug=False,
                   num_devices=N_CORES)

    # qt/kt ship with rows duplicated into partitions 64..127 so two K=64
    # scores matmuls can run concurrently in the PE array's two row-halves.
    qt = nc.dram_tensor("qt", [2 * D_K, GPC], bf16, kind="ExternalInput").ap()
    kt = nc.dram_tensor("kt", [2 * D_K, N_LATENT], bf16, kind="ExternalInput").ap()
    vp = nc.dram_tensor("vp", [N_LATENT, N_CELLS], bf16, kind="ExternalInput").ap()
    gt = nc.dram_tensor("gt", [N_LATENT, GPC], f16, kind="ExternalInput").ap()
    pt = nc.dram_tensor("pt", [N_LATENT, GPC], bf16, kind="ExternalOutput").ap()
    outp = nc.dram_tensor("outp", [GPC, N_CELLS], f32, kind="ExternalOutput").ap()

    Exp = mybir.ActivationFunctionType.Exp

    with tile.TileContext(nc) as tc, contextlib.ExitStack() as ctx:
        consts = ctx.enter_context(tc.tile_pool(name="consts", bufs=1))
        gpool = ctx.enter_context(tc.tile_pool(name="gpool", bufs=8))
        epool = ctx.enter_context(tc.tile_pool(name="epool", bufs=2))
        outpool = ctx.enter_context(tc.tile_pool(name="outpool", bufs=3))
        lpool = ctx.enter_context(tc.tile_pool(name="lpool", bufs=4))
        scpool = ctx.enter_context(tc.tile_pool(name="scpool", bufs=2, space="PSUM"))
        popool = ctx.enter_context(tc.tile_pool(name="popool", bufs=2, space="PSUM"))

        # Load only what the first matmuls need up front; the rest of the
        # constant loads are deferred into block 0's step loop so the
        # critical first chunks don't contend for DMA bandwidth.
        qt_s = consts.tile([2 * D_K, GPC], bf16)
        nc.sync.dma_start(out=qt_s[:, 0:GB], in_=qt[:, 0:GB])
        kt_s = consts.tile([2 * D_K, N_LATENT], bf16)
        nc.sync.dma_start(out=kt_s[:, 0:256], in_=kt[:, 0:256])
        nc.sync.dma_start(out=kt_s[:, 256:GB], in_=kt[:, 256:GB])
        vp_s = consts.tile([128, NLT, N_CELLS], bf16)
        vp_r = vp.rearrange("(t p) c -> p t c", p=128)

        def deferred_const_loads():
            # kt chunk j is needed by packed step 2j -- load those first;
            # qt chunks are only needed from block 1 onward.
            for j in range(1, N_LATENT // GB):
                yield lambda j=j: nc.sync.dma_start(
                    out=kt_s[:, j * GB:(j + 1) * GB], in_=kt[:, j * GB:(j + 1) * GB])
            for j in range(1, NBLK):
                yield lambda j=j: nc.sync.dma_start(
                    out=qt_s[:, j * GB:(j + 1) * GB], in_=qt[:, j * GB:(j + 1) * GB])

        _deferred = deferred_const_loads()

        exps_t = [None] * NBLK
        PREF = 2  # gt prefetch distance, in packed (2-tile) steps

        def phase_a_steps(b):
            """Packed steps for block b, two latent tiles at a time:
            noise DMA (prefetched), 2 concurrent scores MMs (row-halves
            of the PE array), +noise (DVE), exp (ACT), expT DMA out (on
            GpSimd queues -- Sync is the busier DMA issuer)."""
            exps = epool.tile([128, NLT, GB], bf16, name="exps")
            exps_t[b] = exps
            gtiles = {}

            def fetch(k):
                if k >= NLT // 2:
                    return
                gti = gpool.tile([128, 2 * GB], f16, name="gti")
                i = 2 * k
                nc.sync.dma_start(
                    out=gti[:, 0:GB],
                    in_=gt[i * 128:(i + 1) * 128, b * GB:(b + 1) * GB])
                nc.sync.dma_start(
                    out=gti[:, GB:2 * GB],
                    in_=gt[(i + 1) * 128:(i + 2) * 128, b * GB:(b + 1) * GB])
                gtiles[k] = gti

            for k in range(PREF):
                fetch(k)
            for k in range(NLT // 2):
                i = 2 * k
                fetch(k + PREF)
                if b == 0:
                    # hide the remaining const loads + V preload behind
                    # block 0's noise stream
                    for fn in (next(_deferred, None), next(_deferred, None)):
                        if fn is not None:
                            fn()
                    nc.sync.dma_start(out=vp_s[:, i, :], in_=vp_r[:, i, :])
                    nc.sync.dma_start(out=vp_s[:, i + 1, :], in_=vp_r[:, i + 1, :])
                gti = gtiles.pop(k)
                sc = scpool.tile([128, 2 * GB], f32, name="sc")
                nc.tensor.matmul(
                    sc[:, 0:GB], lhsT=kt_s[0:D_K, i * 128:(i + 1) * 128],
                    rhs=qt_s[0:D_K, b * GB:(b + 1) * GB], start=True, stop=True)
                nc.tensor.matmul(
                    sc[:, GB:2 * GB],
                    lhsT=kt_s[D_K:2 * D_K, (i + 1) * 128:(i + 2) * 128],
                    rhs=qt_s[D_K:2 * D_K, b * GB:(b + 1) * GB],
                    start=True, stop=True)
                es = lpool.tile([128, 2 * GB], f32, name="es")
                nc.scalar.activation(out=es, in_=sc, func=Exp)
                # exp(s+g) = exp(s) * expg; alternate DVE / GpSimd
                eng = nc.vector if k % 2 == 0 else nc.gpsimd
                eng.tensor_mul(exps[:, i:i + 2, :], es, gti)
                pt_view = pt[i * 128:(i + 2) * 128, b * GB:(b + 1) * GB] \
                    .rearrange("(t p) g -> p t g", p=128)
                nc.gpsimd.dma_start(out=pt_view, in_=exps[:, i:i + 2, :])
                yield

        def out_mms(b, s, i, po):
            lhs = exps_t[b][:, i, s * 128:(s + 1) * 128]
            nc.tensor.matmul(po[:, 0:512], lhsT=lhs, rhs=vp_s[:, i, 0:512],
                             start=(i == 0), stop=(i == NLT - 1))
            nc.tensor.matmul(po[:, 512:1024], lhsT=lhs, rhs=vp_s[:, i, 512:1024],
                             start=(i == 0), stop=(i == NLT - 1))

        def emit_outn(b, s, po):
            outn = outpool.tile([128, N_CELLS], f32, name="outn")
            nc.scalar.copy(out=outn[:, 0:512], in_=po[:, 0:512])
            nc.vector.tensor_copy(out=outn[:, 512:1024], in_=po[:, 512:1024])
            g0 = (b * NSB + s) * 128
            nc.sync.dma_start(out=outp[g0:g0 + 128, :], in_=outn)

        def emit_phase_b(b, subs, a_next, period):
            """out_un = expT.T @ V per 128-gene sub-block, with the next
            block's phase-A steps interleaved so DVE/ACT chew block b+1
            while PE runs block b's matmuls."""
            for s in subs:
                po = popool.tile([128, N_CELLS], f32, name="po")
                for i in range(NLT):
                    out_mms(b, s, i, po)
                    if a_next is not None and i % period == 1:
                        next(a_next, None)
                emit_outn(b, s, po)
            if a_next is not None:
                for _ in a_next:
                    pass

        # Software pipeline.  Block 0's phase A is DMA-bound (noise + V
        # preload) with PE mostly idle, so sub-block 0's out-matmuls are
        # interleaved directly into it, lagging the exp stream by 2 steps.
        a0 = phase_a_steps(0)
        next(a0)
        next(a0)
        po0 = popool.tile([128, N_CELLS], f32, name="po")
        for k in range(2, NLT // 2):
            next(a0, None)
            for i in (2 * (k - 2), 2 * (k - 2) + 1):
                out_mms(0, 0, i, po0)
        for _ in a0:
            pass
        for i in range(NLT - 4, NLT):
            out_mms(0, 0, i, po0)
        emit_outn(0, 0, po0)

        emit_phase_b(0, range(1, NSB), phase_a_steps(1), period=6)
        for b in range(1, NBLK):
            a_next = phase_a_steps(b + 1) if b + 1 < NBLK else None
            emit_phase_b(b, range(NSB), a_next, period=8)

    nc.compile()
    return nc


def _prep_inputs(query, key, value):
    import ml_dtypes

    bf16 = ml_dtypes.bfloat16
    if "gt" not in _CACHE:
        g = _gumbel_noise()
        # ship exp(g)*2^-8 in fp16 (exp-domain: uniform 2^-11 relative
        # error; the 2^-8 scale keeps it in fp16 range and cancels in the
        # host-side softmax normalization)
        egt = np.exp(g.T - np.float32(8 * np.log(2))).astype(np.float16)
        _CACHE["gt"] = [np.ascontiguousarray(egt[:, c * GPC:(c + 1) * GPC])
                        for c in range(N_CORES)]
    qt_full = (query.astype(np.float32).T / 8.0).astype(bf16)   # [64, 16384]
    qt_full = np.vstack([qt_full, qt_full])                      # [128, 16384]
    kt1 = np.ascontiguousarray(key.astype(np.float32).T).astype(bf16)
    kt = np.vstack([kt1, kt1])                                   # [128, 4096]
    vp = value.astype(bf16)

    in_maps = []
    for c in range(N_CORES):
        sl = slice(c * GPC, (c + 1) * GPC)
        in_maps.append({
            "qt": np.ascontiguousarray(qt_full[:, sl]),
            "kt": kt,
            "vp": vp,
            "gt": _CACHE["gt"][c],
        })
    return in_maps


LAST_RESULT = None


def _ensure_ntff_hook():
    """Make trace=True usable even when the image lacks antenv.axon_hooks
    (recreate the module and register the ctypes NTFF hook)."""
    import sys, types
    try:
        import antenv.axon_hooks  # noqa: F401
        return
    except ImportError:
        pass
    try:
        import antenv
        from trn_agent_boot.trn_boot import _ntff_profile_via_ctypes
        mod = types.ModuleType("antenv.axon_hooks")
        _h = [None]
        mod.set_axon_ntff_profile_hook = lambda h: _h.__setitem__(0, h)
        mod.get_axon_ntff_profile_hook = lambda: _h[0]
        sys.modules["antenv.axon_hooks"] = mod
        antenv.axon_hooks = mod
        mod.set_axon_ntff_profile_hook(
            _ntff_profile_via_ctypes("/opt/axon/libaxon_pjrt.so"))
    except Exception:
        pass


def kernel(query, key, value):
    global LAST_RESULT
    from concourse.bass_utils import run_bass_kernel_spmd

    trace = bool(int(os.environ.get("KERNEL_TRACE", "0")))
    if trace:
        _ensure_ntff_hook()
    if "nc" not in _CACHE:
        _CACHE["nc"] = _build_bass()
    nc = _CACHE["nc"]

    in_maps = _prep_inputs(np.asarray(query), np.asarray(key), np.asarray(value))
    res = run_bass_kernel_spmd(
        nc, in_maps, core_ids=list(range(N_CORES)), trace=trace)
    LAST_RESULT = res

    out = np.empty((N_GENES, N_CELLS), np.float32)
    p = np.empty((N_GENES, N_LATENT), np.float32)
    for c in range(N_CORES):
        sl = slice(c * GPC, (c + 1) * GPC)
        r = res.results[c]
        ptf = r["pt"].astype(np.float32)          # [latent, genes]
        recip = 1.0 / ptf.sum(axis=0)             # softmax denominators
        out[sl] = r["outp"] * recip[:, None]
        # p[g, l] = expT[l, g] * recip[g]
        p[sl] = (ptf * recip[None, :]).T
    return out, p
